# revision 61
# baseline (speedup 1.0000x reference)
"""Trainium2 Bass kernel for nn_CostVolume: H-sharded across 8 NeuronCores.

v3: bf16 + full-width PE packing (hardware-verified, rel err 5.1e-3).
- BN folded into conv weights on host; all tensors bf16 (fp32 PSUM/bias).
- down(): 1x1 conv K=1024, M=128 = [w1|w1]; dual ACT evac writes Lp/Rp top
  half plus a 1-column-left-shifted bottom half (partitions 64:128), giving
  a free K=128 packing of adjacent kw/s taps for the phase-2 convs.
- conv3a: precomputed 2D convs G_L/G_R/E/F, M-packed to 128 partitions:
  set0 = [var0|var0], set1 = [var2|var1], so the per-d assembly (DVE adds +
  Pool relu) can evacuate either parity half; GL/GR/F fuse tap pairs via the
  shifted Lp/Rp bottom halves (GL 9->6, GR 15->9, F taps packed).
- A-slices live in natural pair tiles R_m = [A[2m-1](p0:64); A[2m](p64:128)]
  written in place by Pool-engine relu evacs -- zero ring-copy DMA, and the
  ACT queue carries only output evacs (no cross-engine queue parking).
- conv3b: per output pair (d,d+1), per tap (kh,kw): 2 full 128x128 matmuls
  (moving R_m, R_{m+1}) -> two single-bank PSUM [128,3,160] tiles (a
  [128,6,160] tile would cross the 2KB PSUM bank boundary and silently
  corrupt accumulation on hardware); 36 matmuls/pair vs 60 in v1.
- Band corrections fused into one diagonal-AP DVE sub per d; final
  relu(x + b3b) applied on host (output evac is a plain Identity copy).
Each core computes 6 output rows (48 d x 64 ch x 6 h x 160 w).
TimelineSim: 234.8us vs 405.4us for v1 (1.73x).
"""

import sys

sys.path.insert(0, "/opt/trn_rl_repo")

import numpy as np
import concourse.bass as bass
import concourse.bacc as bacc
import concourse.mybir as mybir
from concourse import tile

F32 = mybir.dt.float32
BF16 = mybir.dt.bfloat16
RELU = mybir.ActivationFunctionType.Relu
IDENT = mybir.ActivationFunctionType.Identity

H, W, D, CF, CIN = 48, 160, 48, 64, 1024
EPS = 1e-5
NC = 8
HLOC = 6
ROWS_IN = 10  # input rows incl 2-halo each side
ROWS_A = 8  # A rows (out rows +-1)
GVLO = -48  # G_R col range [GVLO, 160)
GW = 208
EV0 = 112  # E col range [112, 160)
EW = 48
FW = 52  # F col range [0, 52) per slot
WP = 162  # padded width of ring tiles
NEG = -1.0e30

KDSETS = {0: (0, 1, 2), 1: (1, 2), 2: (0, 1)}
SET_VARS = [(0, 0), (2, 1)]  # (top var, bottom var) per packed set
NRING = 10  # ring depth (pair tiles R_m); extra slots give WAR slack
NTT = 6  # tt assembly buffers


def _f_taps(var, u):
    kds = KDSETS[var]
    return [kw for kw in range(3) if any(kd > u + kw for kd in kds)]


# static tap lists for F: F_SET_TAPS[set][slot(u+2)] = [(kh, kw)];
# F_SET_TAPS2 packs kw0|kw1 into one K=128 matmul (wh=0) and kw2 alone (wh=1)
F_SET_TAPS = []
F_SET_TAPS2 = []
for _st, (_vt, _vb) in enumerate(SET_VARS):
    slots = []
    slots2 = []
    for _u in range(-2, 2):
        kws = sorted(set(_f_taps(_vt, _u)) | set(_f_taps(_vb, _u)))
        slots.append([(kh, kw) for kh in range(3) for kw in kws])
        whs = ([0] if (0 in kws or 1 in kws) else []) + ([1] if 2 in kws else [])
        slots2.append([(kh, wh) for kh in range(3) for wh in whs])
    F_SET_TAPS.append(slots)
    F_SET_TAPS2.append(slots2)


def _fold_bn(w, b, g, beta, m, v):
    s = (g / np.sqrt(v + EPS)).astype(np.float32)
    return (w * s.reshape(-1, *([1] * (w.ndim - 1)))).astype(np.float32), (
        (b - m) * s + beta
    ).astype(np.float32)


def _bcast0(ap, n):
    """Append a step-0 dim of count n to an AP (free-dim broadcast)."""
    return bass.AP(ap.tensor, ap.offset, list(ap.ap) + [[0, n]])


def _diag(ap_1col, row_stride, diag_stride, cnt):
    """Rows x diagonal AP: [[row_stride, 8], [diag_stride, cnt]] from a
    [128, 8, 1] slice anchored at the first diagonal element."""
    base = list(ap_1col.ap)
    return bass.AP(ap_1col.tensor, ap_1col.offset,
                   [base[0], [row_stride, ROWS_A], [diag_stride, cnt]])


def build_nc():
    nc = bacc.Bacc("TRN2", target_bir_lowering=False, debug=False, num_devices=NC)

    xl_d = nc.dram_tensor("xl", [CIN, ROWS_IN * W], BF16, kind="ExternalInput")
    xr_d = nc.dram_tensor("xr", [CIN, ROWS_IN * W], BF16, kind="ExternalInput")
    w1t_d = nc.dram_tensor("w1t", [128, 8, 128], BF16, kind="ExternalInput")
    wgl_d = nc.dram_tensor("wgl", [128, 2, 6, 128], BF16, kind="ExternalInput")
    wgr_d = nc.dram_tensor("wgr", [128, 2, 9, 128], BF16, kind="ExternalInput")
    we_d = nc.dram_tensor("we", [CF, 2, 9, 128], BF16, kind="ExternalInput")
    wf_d = nc.dram_tensor("wf", [128, 2, 4, 6, 128], BF16, kind="ExternalInput")
    ws_d = nc.dram_tensor("ws", [128, 2, 9, 128], BF16, kind="ExternalInput")
    b1_d = nc.dram_tensor("b1c", [128, 1], F32, kind="ExternalInput")
    b3a_d = nc.dram_tensor("b3a2", [128, 1], F32, kind="ExternalInput")
    rowm_d = nc.dram_tensor("rowm", [128, ROWS_IN], BF16, kind="ExternalInput")
    grm_d = nc.dram_tensor("grm", [128, ROWS_A], F32, kind="ExternalInput")
    y_d = nc.dram_tensor("y", [D, CF, HLOC, W], F32, kind="ExternalOutput")

    with tile.TileContext(nc) as tc:
        with (
            tc.tile_pool(name="wpool", bufs=1) as wpool,
            tc.tile_pool(name="xpool", bufs=1) as xpool,
            tc.tile_pool(name="big", bufs=1) as big,
            tc.tile_pool(name="ost", bufs=3) as ostp,
            tc.tile_pool(name="psg", bufs=2, space="PSUM") as psg_p,
            tc.tile_pool(name="ps3", bufs=6, space="PSUM") as ps3_p,
        ):
            # ---- persistent tiles ----
            Lp = big.tile([128, ROWS_IN, WP], BF16)
            Rp = big.tile([128, ROWS_IN, 212], BF16)
            Gl = big.tile([128, 2, ROWS_A, W], BF16)
            Gr = big.tile([128, 2, ROWS_A, GW], BF16)
            Et = big.tile([128, ROWS_A, 2, EW], BF16)
            Ft = big.tile([128, 2, ROWS_A, 4 * FW], BF16)
            ring = [
                big.tile([128, ROWS_A, WP], BF16, tag=f"S{j}", name=f"S{j}")
                for j in range(NRING)
            ]
            tts = [
                big.tile([128, ROWS_A, W], BF16, tag=f"t{j}", name=f"t{j}")
                for j in range(NTT)
            ]

            # ---- pad memsets (no deps, run first) ----
            nc.vector.memset(Lp[0:64, :, 0:1], 0)
            nc.vector.memset(Lp[0:64, :, 161:162], 0)
            nc.vector.memset(Lp[64:128, :, 160:162], 0)
            nc.vector.memset(Rp[0:64, :, 0:50], 0)
            nc.vector.memset(Rp[64:128, :, 0:49], 0)
            nc.vector.memset(Rp[0:64, :, 210:212], 0)
            nc.vector.memset(Rp[64:128, :, 209:212], 0)
            for j in range(NRING):
                nc.vector.memset(ring[j][:, :, 0:1], 0)
                nc.vector.memset(ring[j][:, :, 161:162], 0)
            nc.vector.memset(ring[0][0:64, :, 1:161], 0)  # A[-1] = 0

            # ---- input + weight DMAs (order = HWDGE/transfer order) ----
            xin = []  # [side][chunk]: rows [0:2), [2:6), [6:10)
            for side in (0, 1):
                t0 = xpool.tile([128, 8, 2, W], BF16, tag=f"xs{side}", name=f"x{side}a")
                t1 = xpool.tile([128, 8, 4, W], BF16, tag=f"xm{side}", name=f"x{side}b")
                t2 = xpool.tile([128, 8, 4, W], BF16, tag=f"xe{side}", name=f"x{side}c")
                xin.append((t0, t1, t2))
            w1t = wpool.tile([128, 8, 128], BF16)
            b1 = wpool.tile([128, 1], F32)
            rowm = wpool.tile([128, ROWS_IN], BF16)
            wgl = wpool.tile([128, 2, 6, 128], BF16)
            wgr = wpool.tile([128, 2, 9, 128], BF16)
            wE = wpool.tile([CF, 2, 9, 128], BF16)
            wf = wpool.tile([128, 2, 4, 6, 128], BF16)
            wS = wpool.tile([128, 2, 9, 128], BF16)
            b3a = wpool.tile([128, 1], F32)
            grm = wpool.tile([128, ROWS_A], F32)

            def _xdma(side, chunk):
                x_d = xl_d if side == 0 else xr_d
                c0, c1, r = [(0, 320, 2), (320, 960, 4), (960, 1600, 4)][chunk]
                nc.sync.dma_start(
                    xin[side][chunk][:],
                    x_d[:, c0:c1].rearrange("(k p) (r c) -> p k r c", p=128, r=r),
                )

            _xdma(0, 0)
            nc.sync.dma_start(w1t[:], w1t_d[:])
            nc.sync.dma_start(b1[:], b1_d[:])
            nc.sync.dma_start(rowm[:], rowm_d[:])
            _xdma(0, 1)
            _xdma(0, 2)
            nc.sync.dma_start(wgl[:], wgl_d[:])
            nc.sync.dma_start(wf[:], wf_d[:])
            _xdma(1, 0)
            nc.sync.dma_start(wgr[:], wgr_d[:])
            _xdma(1, 1)
            nc.sync.dma_start(wE[:], we_d[:])
            nc.sync.dma_start(grm[:], grm_d[:])
            _xdma(1, 2)
            nc.sync.dma_start(wS[:], ws_d[:])
            nc.sync.dma_start(b3a[:], b3a_d[:])

            # ---- phase 1: down() ----
            def phase1(side):
                for c5 in range(5):
                    if c5 == 0:
                        src, rl = xin[side][0], 0
                    elif c5 < 3:
                        src, rl = xin[side][1], 2 * (c5 - 1)
                    else:
                        src, rl = xin[side][2], 2 * (c5 - 3)
                    ps = psg_p.tile([128, 2, GW], F32, tag="psg", name="psd")
                    for k in range(8):
                        nc.tensor.matmul(
                            ps[:, :, 0:W],
                            w1t[:, k, :],
                            src[:, k, rl : rl + 2, :],
                            start=(k == 0),
                            stop=(k == 7),
                        )
                    # dual evac: bottom half lands one column left, giving a
                    # built-in shifted copy for K=128 tap packing in GL/GR/F
                    r = 2 * c5
                    if side == 0:
                        nc.scalar.activation(
                            Lp[0:64, r : r + 2, 1:161], ps[0:64, :, 0:W],
                            RELU, bias=b1[0:64],
                        )
                        nc.scalar.activation(
                            Lp[64:128, r : r + 2, 0:160], ps[64:128, :, 0:W],
                            RELU, bias=b1[64:128],
                        )
                        nc.vector.tensor_mul(
                            Lp[:, r : r + 2, 0:161], Lp[:, r : r + 2, 0:161],
                            _bcast0(rowm[:, r : r + 2], 161),
                        )
                    else:
                        nc.scalar.activation(
                            Rp[0:64, r : r + 2, 50:210], ps[0:64, :, 0:W],
                            RELU, bias=b1[0:64],
                        )
                        nc.scalar.activation(
                            Rp[64:128, r : r + 2, 49:209], ps[64:128, :, 0:W],
                            RELU, bias=b1[64:128],
                        )
                        nc.vector.tensor_mul(
                            Rp[:, r : r + 2, 49:210], Rp[:, r : r + 2, 49:210],
                            _bcast0(rowm[:, r : r + 2], 161),
                        )


            # ---- phase 2: G_L / G_R / E / F (set0=[var0|var0], set1=[var2|var1]) ----
            def phase2L(st):  # L-side convs: only need Lp
                for c4 in range(4):
                    r = 2 * c4
                    ps = psg_p.tile([128, 2, GW], F32, tag="psg", name="psgl")
                    for i6 in range(6):
                        kh, wh = divmod(i6, 2)  # wh0: kw0|kw1 packed, wh1: kw2
                        mb = 0 if wh == 0 else 2
                        nc.tensor.matmul(
                            ps[:, :, 0:W],
                            wgl[:, st, i6, :],
                            Lp[:, r + kh : r + kh + 2, mb : mb + W],
                            start=(i6 == 0),
                            stop=(i6 == 5),
                        )
                    nc.scalar.activation(
                        Gl[:, st, r : r + 2, :], ps[:, :, 0:W], IDENT
                    )
                # F: 4 slot-groups into one PSUM tile (disjoint cols)
                for c4 in range(4):
                    r = 2 * c4
                    ps = psg_p.tile([128, 2, GW], F32, tag="psg", name="psf")
                    for slot in range(4):
                        taps = F_SET_TAPS2[st][slot]
                        for j, (kh, wh) in enumerate(taps):
                            mb = 0 if wh == 0 else 2
                            nc.tensor.matmul(
                                ps[:, :, FW * slot : FW * slot + FW],
                                wf[:, st, slot, 2 * kh + wh, :],
                                Lp[:, r + kh : r + kh + 2, mb : mb + FW],
                                start=(j == 0),
                                stop=(j == len(taps) - 1),
                            )
                    nc.scalar.activation(Ft[:, st, r : r + 2, :], ps[:], IDENT)

            def phase2R(st):  # R-side convs: need Rp
                for c4 in range(4):
                    r = 2 * c4
                    ps = psg_p.tile([128, 2, GW], F32, tag="psg", name="psgr")
                    for i9 in range(9):
                        kh, sp = divmod(i9, 3)  # sp: (s=-2,-1) (0,1) (2,-)
                        nc.tensor.matmul(
                            ps[:],
                            wgr[:, st, 3 * kh + sp, :],
                            Rp[:, r + kh : r + kh + 2, 2 * sp : 2 * sp + GW],
                            start=(i9 == 0),
                            stop=(i9 == 8),
                        )
                    # evac + grmask(-1e30 out-of-image rows) via DVE add
                    nc.vector.tensor_add(
                        Gr[:, st, r : r + 2, :], ps[:], _bcast0(grm[:, r : r + 2], GW)
                    )
                for c4 in range(4):
                    r = 2 * c4
                    ps = psg_p.tile([128, 2, GW], F32, tag="psg", name="pse")
                    for i9 in range(9):
                        kd, kh = divmod(i9, 3)
                        nc.tensor.matmul(
                            ps[:, :, 0:EW],
                            wE[:, st, i9, :],
                            Rp[0:64, r + kh : r + kh + 2, 164 - kd : 164 - kd + EW],
                            start=(i9 == 0),
                            stop=(i9 == 8),
                        )
                    nc.scalar.activation(
                        Et[:, r : r + 2, st, :], ps[:, :, 0:EW], IDENT
                    )

            # ---- phase 3: d-loop ----
            # Ft element strides: [2, ROWS_A, 4*FW] -> row stride 4*FW, col 1
            FT_ROW = 4 * FW
            ET_ROW = 2 * EW

            def produce(i):
                """Assemble A[i] into its ring half (or memset for i=48)."""
                if i > D:
                    return
                m_w = (i + 1) // 2
                Rt = ring[m_w % NRING]
                if i == D:  # A[48] = 0 (bottom of R_24)
                    nc.vector.memset(Rt[64:128, :, 1:161], 0)
                    return
                st = 1 if i in (0, D - 1) else 0
                bandlo = max(0, i - 2)
                tt = tts[i % NTT]
                nc.vector.tensor_add(
                    tt[:, :, bandlo:W],
                    Gl[:, st, :, bandlo:W],
                    Gr[:, st, :, bandlo - i - GVLO : W - i - GVLO],
                )
                # fused band corrections: diagonal over (slot, col)
                u0 = max(-2, -i)
                cnt = 2 - u0
                w0 = i + u0
                anchor = Ft[:, st, 0:1, FW * (u0 + 2) + w0 : FW * (u0 + 2) + w0 + 1]
                nc.vector.tensor_sub(
                    tt[:, :, w0 : w0 + cnt],
                    tt[:, :, w0 : w0 + cnt],
                    _diag(anchor, FT_ROW, FW + 1, cnt),
                )
                # right W-edge correction at w=159
                nc.vector.tensor_sub(
                    tt[:, :, W - 1 : W],
                    tt[:, :, W - 1 : W],
                    Et[:, :, st, W - 1 - i - EV0 : W - i - EV0],
                )
                # relu(x + b3a) on the idle Pool engine (SBUF->SBUF is legal
                # for GPSIMD): keeps the production stream off the ACT queue,
                # which carries the per-pair output evacs.
                p0, p1 = (0, 64) if i % 2 == 1 else (64, 128)
                if bandlo > 0:
                    nc.gpsimd.tensor_scalar(
                        Rt[p0:p1, :, 1 : 1 + bandlo],
                        Gr[p0:p1, st, :, -i - GVLO : bandlo - i - GVLO],
                        b3a[p0:p1], 0.0,
                        mybir.AluOpType.add, mybir.AluOpType.max,
                    )
                nc.gpsimd.tensor_scalar(
                    Rt[p0:p1, :, 1 + bandlo : 161],
                    tt[p0:p1, :, bandlo:W],
                    b3a[p0:p1], 0.0,
                    mybir.AluOpType.add, mybir.AluOpType.max,
                )

            # phase order: L-only phase-2 work overlaps the XR input DMAs;
            # set1 first so A[0] (needs only set1) assembles early.
            phase1(0)
            phase2L(1)
            phase1(1)
            phase2R(1)
            produce(0)
            phase2L(0)
            phase2R(0)
            produce(1)
            produce(2)
            for p in range(D // 2):  # pair p: outputs (2p, 2p+1), uses R_p, R_{p+1}
                d = 2 * p
                # next pair's A-slices first: their DVE/ACT work overlaps this
                # pair's matmuls and stays ahead of the in-order queues.
                produce(2 * p + 3)
                produce(2 * p + 4)
                Tlo = ring[p % NRING]
                Thi = ring[(p + 1) % NRING]
                psA = ps3_p.tile([128, 3, W], F32, tag="ps3", name="psA")
                psB = ps3_p.tile([128, 3, W], F32, tag="ps3", name="psB")
                pss = {0: psA, 3: psB}
                # all Tlo taps first (both j0 groups): releases the R_p ring
                # reads at 50% of the pair so the ACT production stream's
                # ring-WAR waits unpark mid-pair, not at pair end.
                for j0 in (0, 3):
                    for i9 in range(9):
                        kh, kw = divmod(i9, 3)
                        nc.tensor.matmul(
                            pss[j0][:],
                            wS[:, 0, i9, :],
                            Tlo[:, j0 + kh : j0 + kh + 3, kw : kw + W],
                            start=(i9 == 0),
                            stop=False,
                        )
                for j0 in (0, 3):
                    for i9 in range(9):
                        kh, kw = divmod(i9, 3)
                        nc.tensor.matmul(
                            pss[j0][:],
                            wS[:, 1, i9, :],
                            Thi[:, j0 + kh : j0 + kh + 3, kw : kw + W],
                            start=False,
                            stop=(i9 == 8),
                        )
                # plain PSUM->SBUF copies (relu + b3b applied on host), then out
                _evac_half(nc, ostp, y_d, psA, d, 0)
                _evac_half(nc, ostp, y_d, psB, d, 3)

    nc.finalize()
    return nc


_NC_CACHE = None


def _evac_half(nc, ostp, y_d, ps, d, j0):
    ost = ostp.tile([128, 3, W], F32, tag="ost", name="ost")
    nc.scalar.activation(ost[:], ps[:], IDENT)
    nc.sync.dma_start(y_d[d : d + 2, :, j0 : j0 + 3, :], ost[:])


def _get_nc():
    global _NC_CACHE
    if _NC_CACHE is None:
        _NC_CACHE = build_nc()
    return _NC_CACHE


def _to_bf16(x):
    import ml_dtypes

    return np.asarray(x, np.float32).astype(ml_dtypes.bfloat16)


def _prep_weights(inputs):
    w1, b1 = _fold_bn(
        inputs["conv1_w"], inputs["conv1_b"], inputs["bn1_g"], inputs["bn1_b"],
        inputs["bn1_m"], inputs["bn1_v"],
    )
    w3a, b3a = _fold_bn(
        inputs["c3a_w"], inputs["c3a_b"], inputs["bn3a_g"], inputs["bn3a_b"],
        inputs["bn3a_m"], inputs["bn3a_v"],
    )
    w3b, b3b = _fold_bn(
        inputs["c3b_w"], inputs["c3b_b"], inputs["bn3b_g"], inputs["bn3b_b"],
        inputs["bn3b_m"], inputs["bn3b_v"],
    )
    wl, wr = w3a[:, :CF], w3a[:, CF:]

    out = {}
    w1t_f = np.ascontiguousarray(w1.T.reshape(8, 128, CF).transpose(1, 0, 2))
    out["w1t"] = _to_bf16(np.concatenate([w1t_f, w1t_f], axis=2))

    wgl_f = np.zeros((CF, 2, 9, 128), np.float32)
    for st, (vt, vb) in enumerate(SET_VARS):
        for half, var in ((0, vt), (1, vb)):
            k = sum(wl[:, :, kd] for kd in KDSETS[var])
            for kh in range(3):
                for kw in range(3):
                    wgl_f[:, st, 3 * kh + kw, 64 * half : 64 * half + 64] = (
                        k[:, :, kh, kw].T
                    )
    # pack: K rows 0:64 = kw_top taps, rows 64:128 = kw_top+1 taps
    wgl2 = np.zeros((128, 2, 6, 128), np.float32)
    for st in range(2):
        for kh in range(3):
            wgl2[0:64, st, 2 * kh + 0, :] = wgl_f[:, st, 3 * kh + 0, :]
            wgl2[64:128, st, 2 * kh + 0, :] = wgl_f[:, st, 3 * kh + 1, :]
            wgl2[0:64, st, 2 * kh + 1, :] = wgl_f[:, st, 3 * kh + 2, :]
    out["wgl"] = _to_bf16(wgl2)

    wgr_f = np.zeros((CF, 2, 15, 128), np.float32)
    for st, (vt, vb) in enumerate(SET_VARS):
        for half, var in ((0, vt), (1, vb)):
            for s in range(-2, 3):
                for kh in range(3):
                    acc = np.zeros((CF, CF), np.float32)
                    for kd in KDSETS[var]:
                        kw = s + kd
                        if 0 <= kw < 3:
                            acc += wr[:, :, kd, kh, kw]
                    wgr_f[:, st, 3 * (s + 2) + kh, 64 * half : 64 * half + 64] = acc.T
    wgr2 = np.zeros((128, 2, 9, 128), np.float32)
    for st in range(2):
        for kh in range(3):
            for sp in range(3):  # s_top = 2*sp - 2
                wgr2[0:64, st, 3 * kh + sp, :] = wgr_f[:, st, 3 * (2 * sp) + kh, :]
                if sp < 2:
                    wgr2[64:128, st, 3 * kh + sp, :] = (
                        wgr_f[:, st, 3 * (2 * sp + 1) + kh, :]
                    )
    out["wgr"] = _to_bf16(wgr2)

    wE = np.zeros((CF, 2, 9, 128), np.float32)
    for st, (vt, vb) in enumerate(SET_VARS):
        for half, var in ((0, vt), (1, vb)):
            for kd in KDSETS[var]:
                for kh in range(3):
                    wE[:, st, 3 * kd + kh, 64 * half : 64 * half + 64] = (
                        wr[:, :, kd, kh, 2].T
                    )
    out["we"] = _to_bf16(wE)

    wf_f = np.zeros((CF, 2, 4, 9, 128), np.float32)
    for st, (vt, vb) in enumerate(SET_VARS):
        for half, var in ((0, vt), (1, vb)):
            for u in range(-2, 2):
                for kh in range(3):
                    for kw in _f_taps(var, u):
                        acc = np.zeros((CF, CF), np.float32)
                        for kd in KDSETS[var]:
                            if kd > u + kw:
                                acc += wl[:, :, kd, kh, kw]
                        wf_f[:, st, u + 2, 3 * kh + kw, 64 * half : 64 * half + 64] = (
                            acc.T
                        )
    wf2 = np.zeros((128, 2, 4, 6, 128), np.float32)
    for st in range(2):
        for slot in range(4):
            for kh in range(3):
                wf2[0:64, st, slot, 2 * kh + 0, :] = wf_f[:, st, slot, 3 * kh + 0, :]
                wf2[64:128, st, slot, 2 * kh + 0, :] = wf_f[:, st, slot, 3 * kh + 1, :]
                wf2[0:64, st, slot, 2 * kh + 1, :] = wf_f[:, st, slot, 3 * kh + 2, :]
    out["wf"] = _to_bf16(wf2)

    wS = np.zeros((128, 2, 9, 128), np.float32)
    for kh in range(3):
        for kw in range(3):
            t = 3 * kh + kw
            wS[0:64, 0, t, 0:64] = w3b[:, :, 0, kh, kw].T
            wS[64:128, 0, t, 0:64] = w3b[:, :, 1, kh, kw].T
            wS[64:128, 0, t, 64:128] = w3b[:, :, 0, kh, kw].T
            wS[0:64, 1, t, 0:64] = w3b[:, :, 2, kh, kw].T
            wS[0:64, 1, t, 64:128] = w3b[:, :, 1, kh, kw].T
            wS[64:128, 1, t, 64:128] = w3b[:, :, 2, kh, kw].T
    out["ws"] = _to_bf16(wS)

    out["b1c"] = np.concatenate([b1, b1]).reshape(128, 1)
    out["b3a2"] = np.concatenate([b3a, b3a]).reshape(128, 1)
    out["b3b2"] = np.concatenate([b3b, b3b]).reshape(128, 1)
    return out


def _per_core_inputs(inputs, shared, c):
    import ml_dtypes

    r0 = 6 * c
    rows = np.arange(r0 - 2, r0 + 8)
    valid = (rows >= 0) & (rows < H)

    def slc(x):
        out = np.zeros((CIN, ROWS_IN, W), np.float32)
        out[:, valid] = x[0][:, rows[valid]]
        return out.reshape(CIN, ROWS_IN * W).astype(ml_dtypes.bfloat16)

    m = dict(shared)
    m["xl"] = slc(np.asarray(inputs["left_features"], np.float32))
    m["xr"] = slc(np.asarray(inputs["right_features"], np.float32))
    m["rowm"] = np.broadcast_to(
        valid.astype(np.float32), (128, ROWS_IN)
    ).astype(ml_dtypes.bfloat16)
    arows = np.arange(r0 - 1, r0 + 7)
    gvals = np.where((arows >= 0) & (arows < H), 0.0, NEG).astype(np.float32)
    m["grm"] = np.broadcast_to(gvals, (128, ROWS_A)).copy()
    return m


_EXEC_CACHE = None


def _get_exec():
    """Build the SPMD executable once; reuse across kernel() calls."""
    global _EXEC_CACHE
    if _EXEC_CACHE is not None:
        return _EXEC_CACHE
    import jax
    import concourse.mybir as mb
    from concourse import bass2jax
    from jax.experimental.shard_map import shard_map
    from jax.sharding import Mesh, PartitionSpec

    nc = _get_nc()
    bass2jax.install_neuronx_cc_hook()
    partition_name = nc.partition_id_tensor.name if nc.partition_id_tensor else None
    in_names, out_names, out_avals, zero_outs = [], [], [], []
    for alloc in nc.m.functions[0].allocations:
        if not isinstance(alloc, mb.MemoryLocationSet):
            continue
        name = alloc.memorylocations[0].name
        if alloc.kind == "ExternalInput":
            if name != partition_name:
                in_names.append(name)
        elif alloc.kind == "ExternalOutput":
            shape = tuple(alloc.tensor_shape)
            dtype = mb.dt.np(alloc.dtype)
            out_names.append(name)
            out_avals.append(jax.core.ShapedArray(shape, dtype))
            zero_outs.append(np.zeros(shape, dtype))
    n_params = len(in_names)
    all_in = list(in_names) + list(out_names)
    if partition_name is not None:
        all_in.append(partition_name)

    def _body(*args):
        operands = list(args)
        if partition_name is not None:
            operands.append(bass2jax.partition_id_tensor())
        outs = bass2jax._bass_exec_p.bind(
            *operands,
            out_avals=tuple(out_avals),
            in_names=tuple(all_in),
            out_names=tuple(out_names),
            lowering_input_output_aliases=(),
            sim_require_finite=True,
            sim_require_nnan=True,
            nc=nc,
        )
        return tuple(outs)

    devices = jax.devices()[:NC]
    mesh = Mesh(np.asarray(devices), ("core",))
    n_outs = len(out_names)
    sharded = jax.jit(
        shard_map(
            _body,
            mesh=mesh,
            in_specs=(PartitionSpec("core"),) * (n_params + n_outs),
            out_specs=(PartitionSpec("core"),) * n_outs,
            check_rep=False,
        ),
        donate_argnums=tuple(range(n_params, n_params + n_outs)),
        keep_unused=True,
    )
    _EXEC_CACHE = (sharded, in_names, out_names, out_avals, zero_outs)
    return _EXEC_CACHE


def _run(in_maps):
    sharded, in_names, out_names, out_avals, zero_outs = _get_exec()
    concat_in = [
        np.concatenate([np.asarray(in_maps[c][nm]) for c in range(NC)], axis=0)
        for nm in in_names
    ]
    concat_zeros = [
        np.zeros((NC * z.shape[0], *z.shape[1:]), z.dtype) for z in zero_outs
    ]
    out_arrs = sharded(*concat_in, *concat_zeros)
    return [
        {
            nm: np.asarray(out_arrs[i]).reshape(NC, *out_avals[i].shape)[c]
            for i, nm in enumerate(out_names)
        }
        for c in range(NC)
    ]


def kernel(**inputs):
    shared = _prep_weights(inputs)
    b3b = shared["b3b2"][0:CF, 0]  # final conv bias, applied on host
    in_maps = [_per_core_inputs(inputs, shared, c) for c in range(NC)]
    results = _run(in_maps)
    full = np.zeros((CF, D, H, W), np.float32)
    for c in range(NC):
        y = results[c]["y"]  # [48, 64, 6, 160] raw conv3b accumulator
        y = np.maximum(y + b3b[None, :, None, None], 0.0)
        full[:, :, 6 * c : 6 * c + 6, :] = y.transpose(1, 0, 2, 3)
    return full.reshape(1, CF * D, H, W)


# revision 62
# speedup vs baseline: 1.0173x; 1.0173x over previous
"""Trainium2 Bass kernel for nn_CostVolume: H-sharded across 8 NeuronCores.

v3: bf16 + full-width PE packing (hardware-verified, rel err 5.1e-3).
- BN folded into conv weights on host; all tensors bf16 (fp32 PSUM/bias).
- down(): 1x1 conv K=1024, M=128 = [w1|w1]; dual ACT evac writes Lp/Rp top
  half plus a 1-column-left-shifted bottom half (partitions 64:128), giving
  a free K=128 packing of adjacent kw/s taps for the phase-2 convs.
- conv3a: precomputed 2D convs G_L/G_R/E/F, M-packed to 128 partitions:
  set0 = [var0|var0], set1 = [var2|var1], so the per-d assembly (DVE adds +
  Pool relu) can evacuate either parity half; GL/GR/F fuse tap pairs via the
  shifted Lp/Rp bottom halves (GL 9->6, GR 15->9, F taps packed).
- A-slices live in natural pair tiles R_m = [A[2m-1](p0:64); A[2m](p64:128)]
  written in place by Pool-engine relu evacs -- zero ring-copy DMA, and the
  ACT queue carries only output evacs (no cross-engine queue parking).
- conv3b: per output pair (d,d+1), per tap (kh,kw): 2 full 128x128 matmuls
  (moving R_m, R_{m+1}) -> two single-bank PSUM [128,3,160] tiles (a
  [128,6,160] tile would cross the 2KB PSUM bank boundary and silently
  corrupt accumulation on hardware); 36 matmuls/pair vs 60 in v1.
- Band corrections fused into one diagonal-AP DVE sub per d; final
  relu(x + b3b) applied on host (output evac is a plain Identity copy).
Each core computes 6 output rows (48 d x 64 ch x 6 h x 160 w).
TimelineSim: 234.8us vs 405.4us for v1 (1.73x).
"""

import sys

sys.path.insert(0, "/opt/trn_rl_repo")

import numpy as np
import concourse.bass as bass
import concourse.bacc as bacc
import concourse.mybir as mybir
from concourse import tile

F32 = mybir.dt.float32
BF16 = mybir.dt.bfloat16
RELU = mybir.ActivationFunctionType.Relu
IDENT = mybir.ActivationFunctionType.Identity

H, W, D, CF, CIN = 48, 160, 48, 64, 1024
EPS = 1e-5
NC = 8
HLOC = 6
ROWS_IN = 10  # input rows incl 2-halo each side
ROWS_A = 8  # A rows (out rows +-1)
GVLO = -48  # G_R col range [GVLO, 160)
GW = 208
EV0 = 112  # E col range [112, 160)
EW = 48
FW = 52  # F col range [0, 52) per slot
WP = 162  # padded width of ring tiles
NEG = -1.0e30

KDSETS = {0: (0, 1, 2), 1: (1, 2), 2: (0, 1)}
SET_VARS = [(0, 0), (2, 1)]  # (top var, bottom var) per packed set
NRING = 10  # ring depth (pair tiles R_m); extra slots give WAR slack
NTT = 6  # tt assembly buffers


def _f_taps(var, u):
    kds = KDSETS[var]
    return [kw for kw in range(3) if any(kd > u + kw for kd in kds)]


# static tap lists for F: F_SET_TAPS[set][slot(u+2)] = [(kh, kw)];
# F_SET_TAPS2 packs kw0|kw1 into one K=128 matmul (wh=0) and kw2 alone (wh=1)
F_SET_TAPS = []
F_SET_TAPS2 = []
for _st, (_vt, _vb) in enumerate(SET_VARS):
    slots = []
    slots2 = []
    for _u in range(-2, 2):
        kws = sorted(set(_f_taps(_vt, _u)) | set(_f_taps(_vb, _u)))
        slots.append([(kh, kw) for kh in range(3) for kw in kws])
        whs = ([0] if (0 in kws or 1 in kws) else []) + ([1] if 2 in kws else [])
        slots2.append([(kh, wh) for kh in range(3) for wh in whs])
    F_SET_TAPS.append(slots)
    F_SET_TAPS2.append(slots2)


def _fold_bn(w, b, g, beta, m, v):
    s = (g / np.sqrt(v + EPS)).astype(np.float32)
    return (w * s.reshape(-1, *([1] * (w.ndim - 1)))).astype(np.float32), (
        (b - m) * s + beta
    ).astype(np.float32)


def _bcast0(ap, n):
    """Append a step-0 dim of count n to an AP (free-dim broadcast)."""
    return bass.AP(ap.tensor, ap.offset, list(ap.ap) + [[0, n]])


def _diag(ap_1col, row_stride, diag_stride, cnt):
    """Rows x diagonal AP: [[row_stride, 8], [diag_stride, cnt]] from a
    [128, 8, 1] slice anchored at the first diagonal element."""
    base = list(ap_1col.ap)
    return bass.AP(ap_1col.tensor, ap_1col.offset,
                   [base[0], [row_stride, ROWS_A], [diag_stride, cnt]])


def build_nc():
    nc = bacc.Bacc("TRN2", target_bir_lowering=False, debug=False, num_devices=NC)

    xl_d = nc.dram_tensor("xl", [CIN, ROWS_IN * W], BF16, kind="ExternalInput")
    xr_d = nc.dram_tensor("xr", [CIN, ROWS_IN * W], BF16, kind="ExternalInput")
    w1t_d = nc.dram_tensor("w1t", [128, 8, 128], BF16, kind="ExternalInput")
    wgl_d = nc.dram_tensor("wgl", [128, 2, 6, 128], BF16, kind="ExternalInput")
    wgr_d = nc.dram_tensor("wgr", [128, 2, 9, 128], BF16, kind="ExternalInput")
    we_d = nc.dram_tensor("we", [CF, 2, 9, 128], BF16, kind="ExternalInput")
    wf_d = nc.dram_tensor("wf", [128, 2, 4, 6, 128], BF16, kind="ExternalInput")
    ws_d = nc.dram_tensor("ws", [128, 2, 9, 128], BF16, kind="ExternalInput")
    b1_d = nc.dram_tensor("b1c", [128, 1], F32, kind="ExternalInput")
    b3a_d = nc.dram_tensor("b3a2", [128, 1], F32, kind="ExternalInput")
    rowm_d = nc.dram_tensor("rowm", [128, ROWS_IN], BF16, kind="ExternalInput")
    grm_d = nc.dram_tensor("grm", [128, ROWS_A], F32, kind="ExternalInput")
    y_d = nc.dram_tensor("y", [D, CF, HLOC, W], F32, kind="ExternalOutput")

    with tile.TileContext(nc) as tc:
        with (
            tc.tile_pool(name="wpool", bufs=1) as wpool,
            tc.tile_pool(name="xpool", bufs=1) as xpool,
            tc.tile_pool(name="big", bufs=1) as big,
            tc.tile_pool(name="ost", bufs=3) as ostp,
            tc.tile_pool(name="psg", bufs=2, space="PSUM") as psg_p,
            tc.tile_pool(name="ps3", bufs=6, space="PSUM") as ps3_p,
        ):
            # ---- persistent tiles ----
            Lp = big.tile([128, ROWS_IN, WP], BF16)
            Rp = big.tile([128, ROWS_IN, 212], BF16)
            Gl = big.tile([128, 2, ROWS_A, W], BF16)
            Gr = big.tile([128, 2, ROWS_A, GW], BF16)
            Et = big.tile([128, ROWS_A, 2, EW], BF16)
            Ft = big.tile([128, 2, ROWS_A, 4 * FW], BF16)
            ring = [
                big.tile([128, ROWS_A, WP], BF16, tag=f"S{j}", name=f"S{j}")
                for j in range(NRING)
            ]
            tts = [
                big.tile([128, ROWS_A, W], BF16, tag=f"t{j}", name=f"t{j}")
                for j in range(NTT)
            ]

            # ---- pad memsets (no deps, run first) ----
            nc.vector.memset(Lp[0:64, :, 0:1], 0)
            nc.vector.memset(Lp[0:64, :, 161:162], 0)
            nc.vector.memset(Lp[64:128, :, 160:162], 0)
            nc.vector.memset(Rp[0:64, :, 0:50], 0)
            nc.vector.memset(Rp[64:128, :, 0:49], 0)
            nc.vector.memset(Rp[0:64, :, 210:212], 0)
            nc.vector.memset(Rp[64:128, :, 209:212], 0)
            for j in range(NRING):
                nc.vector.memset(ring[j][:, :, 0:1], 0)
                nc.vector.memset(ring[j][:, :, 161:162], 0)
            nc.vector.memset(ring[0][0:64, :, 1:161], 0)  # A[-1] = 0

            # ---- input + weight DMAs (order = HWDGE/transfer order) ----
            xin = []  # [side][chunk]: rows [0:2), [2:6), [6:10)
            for side in (0, 1):
                t0 = xpool.tile([128, 8, 2, W], BF16, tag=f"xs{side}", name=f"x{side}a")
                t1 = xpool.tile([128, 8, 4, W], BF16, tag=f"xm{side}", name=f"x{side}b")
                t2 = xpool.tile([128, 8, 4, W], BF16, tag=f"xe{side}", name=f"x{side}c")
                xin.append((t0, t1, t2))
            w1t = wpool.tile([128, 8, 128], BF16)
            b1 = wpool.tile([128, 1], F32)
            rowm = wpool.tile([128, ROWS_IN], BF16)
            wgl = wpool.tile([128, 2, 6, 128], BF16)
            wgr = wpool.tile([128, 2, 9, 128], BF16)
            wE = wpool.tile([CF, 2, 9, 128], BF16)
            wf = wpool.tile([128, 2, 4, 6, 128], BF16)
            wS = wpool.tile([128, 2, 9, 128], BF16)
            b3a = wpool.tile([128, 1], F32)
            grm = wpool.tile([128, ROWS_A], F32)

            def _xdma(side, chunk):
                x_d = xl_d if side == 0 else xr_d
                c0, c1, r = [(0, 320, 2), (320, 960, 4), (960, 1600, 4)][chunk]
                nc.sync.dma_start(
                    xin[side][chunk][:],
                    x_d[:, c0:c1].rearrange("(k p) (r c) -> p k r c", p=128, r=r),
                )

            _xdma(0, 0)
            nc.sync.dma_start(w1t[:], w1t_d[:])
            nc.sync.dma_start(b1[:], b1_d[:])
            nc.sync.dma_start(rowm[:], rowm_d[:])
            _xdma(0, 1)
            _xdma(0, 2)
            nc.sync.dma_start(wgl[:], wgl_d[:])
            nc.sync.dma_start(wf[:], wf_d[:])
            _xdma(1, 0)
            nc.sync.dma_start(wgr[:], wgr_d[:])
            _xdma(1, 1)
            nc.sync.dma_start(wE[:], we_d[:])
            nc.sync.dma_start(grm[:], grm_d[:])
            _xdma(1, 2)
            nc.sync.dma_start(wS[:], ws_d[:])
            nc.sync.dma_start(b3a[:], b3a_d[:])

            # ---- phase 1: down() ----
            def phase1(side):
                for c5 in range(5):
                    if c5 == 0:
                        src, rl = xin[side][0], 0
                    elif c5 < 3:
                        src, rl = xin[side][1], 2 * (c5 - 1)
                    else:
                        src, rl = xin[side][2], 2 * (c5 - 3)
                    ps = psg_p.tile([128, 2, GW], F32, tag="psg", name="psd")
                    for k in range(8):
                        nc.tensor.matmul(
                            ps[:, :, 0:W],
                            w1t[:, k, :],
                            src[:, k, rl : rl + 2, :],
                            start=(k == 0),
                            stop=(k == 7),
                        )
                    # dual evac: bottom half lands one column left, giving a
                    # built-in shifted copy for K=128 tap packing in GL/GR/F
                    r = 2 * c5
                    if side == 0:
                        nc.scalar.activation(
                            Lp[0:64, r : r + 2, 1:161], ps[0:64, :, 0:W],
                            RELU, bias=b1[0:64],
                        )
                        nc.scalar.activation(
                            Lp[64:128, r : r + 2, 0:160], ps[64:128, :, 0:W],
                            RELU, bias=b1[64:128],
                        )
                        nc.vector.tensor_mul(
                            Lp[:, r : r + 2, 0:161], Lp[:, r : r + 2, 0:161],
                            _bcast0(rowm[:, r : r + 2], 161),
                        )
                    else:
                        nc.scalar.activation(
                            Rp[0:64, r : r + 2, 50:210], ps[0:64, :, 0:W],
                            RELU, bias=b1[0:64],
                        )
                        nc.scalar.activation(
                            Rp[64:128, r : r + 2, 49:209], ps[64:128, :, 0:W],
                            RELU, bias=b1[64:128],
                        )
                        nc.vector.tensor_mul(
                            Rp[:, r : r + 2, 49:210], Rp[:, r : r + 2, 49:210],
                            _bcast0(rowm[:, r : r + 2], 161),
                        )


            # ---- phase 2: G_L / G_R / E / F (set0=[var0|var0], set1=[var2|var1]) ----
            def phase2L(st):  # L-side convs: only need Lp
                for c4 in range(4):
                    r = 2 * c4
                    ps = psg_p.tile([128, 2, GW], F32, tag="psg", name="psgl")
                    for i6 in range(6):
                        kh, wh = divmod(i6, 2)  # wh0: kw0|kw1 packed, wh1: kw2
                        mb = 0 if wh == 0 else 2
                        nc.tensor.matmul(
                            ps[:, :, 0:W],
                            wgl[:, st, i6, :],
                            Lp[:, r + kh : r + kh + 2, mb : mb + W],
                            start=(i6 == 0),
                            stop=(i6 == 5),
                        )
                    nc.scalar.activation(
                        Gl[:, st, r : r + 2, :], ps[:, :, 0:W], IDENT
                    )
                # F: 4 slot-groups into one PSUM tile (disjoint cols)
                for c4 in range(4):
                    r = 2 * c4
                    ps = psg_p.tile([128, 2, GW], F32, tag="psg", name="psf")
                    for slot in range(4):
                        taps = F_SET_TAPS2[st][slot]
                        for j, (kh, wh) in enumerate(taps):
                            mb = 0 if wh == 0 else 2
                            nc.tensor.matmul(
                                ps[:, :, FW * slot : FW * slot + FW],
                                wf[:, st, slot, 2 * kh + wh, :],
                                Lp[:, r + kh : r + kh + 2, mb : mb + FW],
                                start=(j == 0),
                                stop=(j == len(taps) - 1),
                            )
                    nc.scalar.activation(Ft[:, st, r : r + 2, :], ps[:], IDENT)

            def phase2R(st):  # R-side convs: need Rp
                for c4 in range(4):
                    r = 2 * c4
                    ps = psg_p.tile([128, 2, GW], F32, tag="psg", name="psgr")
                    for i9 in range(9):
                        kh, sp = divmod(i9, 3)  # sp: (s=-2,-1) (0,1) (2,-)
                        nc.tensor.matmul(
                            ps[:],
                            wgr[:, st, 3 * kh + sp, :],
                            Rp[:, r + kh : r + kh + 2, 2 * sp : 2 * sp + GW],
                            start=(i9 == 0),
                            stop=(i9 == 8),
                        )
                    # evac + grmask(-1e30 out-of-image rows) via DVE add
                    nc.vector.tensor_add(
                        Gr[:, st, r : r + 2, :], ps[:], _bcast0(grm[:, r : r + 2], GW)
                    )
                for c4 in range(4):
                    r = 2 * c4
                    ps = psg_p.tile([128, 2, GW], F32, tag="psg", name="pse")
                    for i9 in range(9):
                        kd, kh = divmod(i9, 3)
                        nc.tensor.matmul(
                            ps[:, :, 0:EW],
                            wE[:, st, i9, :],
                            Rp[0:64, r + kh : r + kh + 2, 164 - kd : 164 - kd + EW],
                            start=(i9 == 0),
                            stop=(i9 == 8),
                        )
                    nc.scalar.activation(
                        Et[:, r : r + 2, st, :], ps[:, :, 0:EW], IDENT
                    )

            # ---- phase 3: d-loop ----
            # Ft element strides: [2, ROWS_A, 4*FW] -> row stride 4*FW, col 1
            FT_ROW = 4 * FW
            ET_ROW = 2 * EW

            def produce(i):
                """Assemble A[i] into its ring half (or memset for i=48)."""
                if i > D:
                    return
                m_w = (i + 1) // 2
                Rt = ring[m_w % NRING]
                if i == D:  # A[48] = 0 (bottom of R_24)
                    nc.vector.memset(Rt[64:128, :, 1:161], 0)
                    return
                st = 1 if i in (0, D - 1) else 0
                bandlo = max(0, i - 2)
                tt = tts[i % NTT]
                nc.vector.tensor_add(
                    tt[:, :, bandlo:W],
                    Gl[:, st, :, bandlo:W],
                    Gr[:, st, :, bandlo - i - GVLO : W - i - GVLO],
                )
                # fused band corrections: diagonal over (slot, col)
                u0 = max(-2, -i)
                cnt = 2 - u0
                w0 = i + u0
                anchor = Ft[:, st, 0:1, FW * (u0 + 2) + w0 : FW * (u0 + 2) + w0 + 1]
                nc.vector.tensor_sub(
                    tt[:, :, w0 : w0 + cnt],
                    tt[:, :, w0 : w0 + cnt],
                    _diag(anchor, FT_ROW, FW + 1, cnt),
                )
                # right W-edge correction at w=159
                nc.vector.tensor_sub(
                    tt[:, :, W - 1 : W],
                    tt[:, :, W - 1 : W],
                    Et[:, :, st, W - 1 - i - EV0 : W - i - EV0],
                )
                # relu(x + b3a) on the idle Pool engine (SBUF->SBUF is legal
                # for GPSIMD): keeps the production stream off the ACT queue,
                # which carries the per-pair output evacs.
                p0, p1 = (0, 64) if i % 2 == 1 else (64, 128)
                if bandlo > 0:
                    nc.gpsimd.tensor_scalar(
                        Rt[p0:p1, :, 1 : 1 + bandlo],
                        Gr[p0:p1, st, :, -i - GVLO : bandlo - i - GVLO],
                        b3a[p0:p1], 0.0,
                        mybir.AluOpType.add, mybir.AluOpType.max,
                    )
                nc.gpsimd.tensor_scalar(
                    Rt[p0:p1, :, 1 + bandlo : 161],
                    tt[p0:p1, :, bandlo:W],
                    b3a[p0:p1], 0.0,
                    mybir.AluOpType.add, mybir.AluOpType.max,
                )

            # phase order: ALL L-only phase-2 work overlaps the XR input
            # DMAs; set1 first so A[0] (needs only set1) assembles early.
            phase1(0)
            phase2L(1)
            phase2L(0)
            phase1(1)
            phase2R(1)
            produce(0)
            phase2R(0)
            produce(1)
            produce(2)
            for p in range(D // 2):  # pair p: outputs (2p, 2p+1), uses R_p, R_{p+1}
                d = 2 * p
                # next pair's A-slices first: their DVE/ACT work overlaps this
                # pair's matmuls and stays ahead of the in-order queues.
                produce(2 * p + 3)
                produce(2 * p + 4)
                Tlo = ring[p % NRING]
                Thi = ring[(p + 1) % NRING]
                psA = ps3_p.tile([128, 3, W], F32, tag="ps3", name="psA")
                psB = ps3_p.tile([128, 3, W], F32, tag="ps3", name="psB")
                pss = {0: psA, 3: psB}
                # all Tlo taps first (both j0 groups): releases the R_p ring
                # reads at 50% of the pair so the ACT production stream's
                # ring-WAR waits unpark mid-pair, not at pair end.
                for j0 in (0, 3):
                    for i9 in range(9):
                        kh, kw = divmod(i9, 3)
                        nc.tensor.matmul(
                            pss[j0][:],
                            wS[:, 0, i9, :],
                            Tlo[:, j0 + kh : j0 + kh + 3, kw : kw + W],
                            start=(i9 == 0),
                            stop=False,
                        )
                for j0 in (0, 3):
                    for i9 in range(9):
                        kh, kw = divmod(i9, 3)
                        nc.tensor.matmul(
                            pss[j0][:],
                            wS[:, 1, i9, :],
                            Thi[:, j0 + kh : j0 + kh + 3, kw : kw + W],
                            start=False,
                            stop=(i9 == 8),
                        )
                # plain PSUM->SBUF copies (relu + b3b applied on host), then out
                _evac_half(nc, ostp, y_d, psA, d, 0)
                _evac_half(nc, ostp, y_d, psB, d, 3)

    nc.finalize()
    return nc


_NC_CACHE = None


def _evac_half(nc, ostp, y_d, ps, d, j0):
    ost = ostp.tile([128, 3, W], F32, tag="ost", name="ost")
    nc.scalar.activation(ost[:], ps[:], IDENT)
    nc.sync.dma_start(y_d[d : d + 2, :, j0 : j0 + 3, :], ost[:])


def _get_nc():
    global _NC_CACHE
    if _NC_CACHE is None:
        _NC_CACHE = build_nc()
    return _NC_CACHE


def _to_bf16(x):
    import ml_dtypes

    return np.asarray(x, np.float32).astype(ml_dtypes.bfloat16)


def _prep_weights(inputs):
    w1, b1 = _fold_bn(
        inputs["conv1_w"], inputs["conv1_b"], inputs["bn1_g"], inputs["bn1_b"],
        inputs["bn1_m"], inputs["bn1_v"],
    )
    w3a, b3a = _fold_bn(
        inputs["c3a_w"], inputs["c3a_b"], inputs["bn3a_g"], inputs["bn3a_b"],
        inputs["bn3a_m"], inputs["bn3a_v"],
    )
    w3b, b3b = _fold_bn(
        inputs["c3b_w"], inputs["c3b_b"], inputs["bn3b_g"], inputs["bn3b_b"],
        inputs["bn3b_m"], inputs["bn3b_v"],
    )
    wl, wr = w3a[:, :CF], w3a[:, CF:]

    out = {}
    w1t_f = np.ascontiguousarray(w1.T.reshape(8, 128, CF).transpose(1, 0, 2))
    out["w1t"] = _to_bf16(np.concatenate([w1t_f, w1t_f], axis=2))

    wgl_f = np.zeros((CF, 2, 9, 128), np.float32)
    for st, (vt, vb) in enumerate(SET_VARS):
        for half, var in ((0, vt), (1, vb)):
            k = sum(wl[:, :, kd] for kd in KDSETS[var])
            for kh in range(3):
                for kw in range(3):
                    wgl_f[:, st, 3 * kh + kw, 64 * half : 64 * half + 64] = (
                        k[:, :, kh, kw].T
                    )
    # pack: K rows 0:64 = kw_top taps, rows 64:128 = kw_top+1 taps
    wgl2 = np.zeros((128, 2, 6, 128), np.float32)
    for st in range(2):
        for kh in range(3):
            wgl2[0:64, st, 2 * kh + 0, :] = wgl_f[:, st, 3 * kh + 0, :]
            wgl2[64:128, st, 2 * kh + 0, :] = wgl_f[:, st, 3 * kh + 1, :]
            wgl2[0:64, st, 2 * kh + 1, :] = wgl_f[:, st, 3 * kh + 2, :]
    out["wgl"] = _to_bf16(wgl2)

    wgr_f = np.zeros((CF, 2, 15, 128), np.float32)
    for st, (vt, vb) in enumerate(SET_VARS):
        for half, var in ((0, vt), (1, vb)):
            for s in range(-2, 3):
                for kh in range(3):
                    acc = np.zeros((CF, CF), np.float32)
                    for kd in KDSETS[var]:
                        kw = s + kd
                        if 0 <= kw < 3:
                            acc += wr[:, :, kd, kh, kw]
                    wgr_f[:, st, 3 * (s + 2) + kh, 64 * half : 64 * half + 64] = acc.T
    wgr2 = np.zeros((128, 2, 9, 128), np.float32)
    for st in range(2):
        for kh in range(3):
            for sp in range(3):  # s_top = 2*sp - 2
                wgr2[0:64, st, 3 * kh + sp, :] = wgr_f[:, st, 3 * (2 * sp) + kh, :]
                if sp < 2:
                    wgr2[64:128, st, 3 * kh + sp, :] = (
                        wgr_f[:, st, 3 * (2 * sp + 1) + kh, :]
                    )
    out["wgr"] = _to_bf16(wgr2)

    wE = np.zeros((CF, 2, 9, 128), np.float32)
    for st, (vt, vb) in enumerate(SET_VARS):
        for half, var in ((0, vt), (1, vb)):
            for kd in KDSETS[var]:
                for kh in range(3):
                    wE[:, st, 3 * kd + kh, 64 * half : 64 * half + 64] = (
                        wr[:, :, kd, kh, 2].T
                    )
    out["we"] = _to_bf16(wE)

    wf_f = np.zeros((CF, 2, 4, 9, 128), np.float32)
    for st, (vt, vb) in enumerate(SET_VARS):
        for half, var in ((0, vt), (1, vb)):
            for u in range(-2, 2):
                for kh in range(3):
                    for kw in _f_taps(var, u):
                        acc = np.zeros((CF, CF), np.float32)
                        for kd in KDSETS[var]:
                            if kd > u + kw:
                                acc += wl[:, :, kd, kh, kw]
                        wf_f[:, st, u + 2, 3 * kh + kw, 64 * half : 64 * half + 64] = (
                            acc.T
                        )
    wf2 = np.zeros((128, 2, 4, 6, 128), np.float32)
    for st in range(2):
        for slot in range(4):
            for kh in range(3):
                wf2[0:64, st, slot, 2 * kh + 0, :] = wf_f[:, st, slot, 3 * kh + 0, :]
                wf2[64:128, st, slot, 2 * kh + 0, :] = wf_f[:, st, slot, 3 * kh + 1, :]
                wf2[0:64, st, slot, 2 * kh + 1, :] = wf_f[:, st, slot, 3 * kh + 2, :]
    out["wf"] = _to_bf16(wf2)

    wS = np.zeros((128, 2, 9, 128), np.float32)
    for kh in range(3):
        for kw in range(3):
            t = 3 * kh + kw
            wS[0:64, 0, t, 0:64] = w3b[:, :, 0, kh, kw].T
            wS[64:128, 0, t, 0:64] = w3b[:, :, 1, kh, kw].T
            wS[64:128, 0, t, 64:128] = w3b[:, :, 0, kh, kw].T
            wS[0:64, 1, t, 0:64] = w3b[:, :, 2, kh, kw].T
            wS[0:64, 1, t, 64:128] = w3b[:, :, 1, kh, kw].T
            wS[64:128, 1, t, 64:128] = w3b[:, :, 2, kh, kw].T
    out["ws"] = _to_bf16(wS)

    out["b1c"] = np.concatenate([b1, b1]).reshape(128, 1)
    out["b3a2"] = np.concatenate([b3a, b3a]).reshape(128, 1)
    out["b3b2"] = np.concatenate([b3b, b3b]).reshape(128, 1)
    return out


def _per_core_inputs(inputs, shared, c):
    import ml_dtypes

    r0 = 6 * c
    rows = np.arange(r0 - 2, r0 + 8)
    valid = (rows >= 0) & (rows < H)

    def slc(x):
        out = np.zeros((CIN, ROWS_IN, W), np.float32)
        out[:, valid] = x[0][:, rows[valid]]
        return out.reshape(CIN, ROWS_IN * W).astype(ml_dtypes.bfloat16)

    m = dict(shared)
    m["xl"] = slc(np.asarray(inputs["left_features"], np.float32))
    m["xr"] = slc(np.asarray(inputs["right_features"], np.float32))
    m["rowm"] = np.broadcast_to(
        valid.astype(np.float32), (128, ROWS_IN)
    ).astype(ml_dtypes.bfloat16)
    arows = np.arange(r0 - 1, r0 + 7)
    gvals = np.where((arows >= 0) & (arows < H), 0.0, NEG).astype(np.float32)
    m["grm"] = np.broadcast_to(gvals, (128, ROWS_A)).copy()
    return m


_EXEC_CACHE = None


def _get_exec():
    """Build the SPMD executable once; reuse across kernel() calls."""
    global _EXEC_CACHE
    if _EXEC_CACHE is not None:
        return _EXEC_CACHE
    import jax
    import concourse.mybir as mb
    from concourse import bass2jax
    from jax.experimental.shard_map import shard_map
    from jax.sharding import Mesh, PartitionSpec

    nc = _get_nc()
    bass2jax.install_neuronx_cc_hook()
    partition_name = nc.partition_id_tensor.name if nc.partition_id_tensor else None
    in_names, out_names, out_avals, zero_outs = [], [], [], []
    for alloc in nc.m.functions[0].allocations:
        if not isinstance(alloc, mb.MemoryLocationSet):
            continue
        name = alloc.memorylocations[0].name
        if alloc.kind == "ExternalInput":
            if name != partition_name:
                in_names.append(name)
        elif alloc.kind == "ExternalOutput":
            shape = tuple(alloc.tensor_shape)
            dtype = mb.dt.np(alloc.dtype)
            out_names.append(name)
            out_avals.append(jax.core.ShapedArray(shape, dtype))
            zero_outs.append(np.zeros(shape, dtype))
    n_params = len(in_names)
    all_in = list(in_names) + list(out_names)
    if partition_name is not None:
        all_in.append(partition_name)

    def _body(*args):
        operands = list(args)
        if partition_name is not None:
            operands.append(bass2jax.partition_id_tensor())
        outs = bass2jax._bass_exec_p.bind(
            *operands,
            out_avals=tuple(out_avals),
            in_names=tuple(all_in),
            out_names=tuple(out_names),
            lowering_input_output_aliases=(),
            sim_require_finite=True,
            sim_require_nnan=True,
            nc=nc,
        )
        return tuple(outs)

    devices = jax.devices()[:NC]
    mesh = Mesh(np.asarray(devices), ("core",))
    n_outs = len(out_names)
    sharded = jax.jit(
        shard_map(
            _body,
            mesh=mesh,
            in_specs=(PartitionSpec("core"),) * (n_params + n_outs),
            out_specs=(PartitionSpec("core"),) * n_outs,
            check_rep=False,
        ),
        donate_argnums=tuple(range(n_params, n_params + n_outs)),
        keep_unused=True,
    )
    _EXEC_CACHE = (sharded, in_names, out_names, out_avals, zero_outs)
    return _EXEC_CACHE


def _run(in_maps):
    sharded, in_names, out_names, out_avals, zero_outs = _get_exec()
    concat_in = [
        np.concatenate([np.asarray(in_maps[c][nm]) for c in range(NC)], axis=0)
        for nm in in_names
    ]
    concat_zeros = [
        np.zeros((NC * z.shape[0], *z.shape[1:]), z.dtype) for z in zero_outs
    ]
    out_arrs = sharded(*concat_in, *concat_zeros)
    return [
        {
            nm: np.asarray(out_arrs[i]).reshape(NC, *out_avals[i].shape)[c]
            for i, nm in enumerate(out_names)
        }
        for c in range(NC)
    ]


def kernel(**inputs):
    shared = _prep_weights(inputs)
    b3b = shared["b3b2"][0:CF, 0]  # final conv bias, applied on host
    in_maps = [_per_core_inputs(inputs, shared, c) for c in range(NC)]
    results = _run(in_maps)
    full = np.zeros((CF, D, H, W), np.float32)
    for c in range(NC):
        y = results[c]["y"]  # [48, 64, 6, 160] raw conv3b accumulator
        y = np.maximum(y + b3b[None, :, None, None], 0.0)
        full[:, :, 6 * c : 6 * c + 6, :] = y.transpose(1, 0, 2, 3)
    return full.reshape(1, CF * D, H, W)
